# revision 23
# baseline (speedup 1.0000x reference)
"""Trainium2 Bass kernel for nn_DPConv_16011638080100.

8 NeuronCores, data-parallel over batch N=16 -> 2 samples/core.

Per core:
  - pooling commutes with 1x1 conv1: pool x first (separable 2-tap averages;
    edge-copy rows done as self-adds so a uniform 0.25 folds into W1a)
  - conv1a on pooled windows -> zero-ring-padded window tiles
  - conv2 + pos 3x3 as 9 PSUM-accumulated matmul taps (center fused w/ conv2);
    tap weights are block-diagonal [128,128] so one matmul covers both
    windows of the round (pt holds window A on partitions 0-63, B on 64-127)
  - SE reduce fused into the PSUM eviction via activation(accum_out=...)
  - torch-.view fold == index permutation
      out[4a+b, 32i+2p+q, 32j+2r+s] = att_a[16b+p, 16q+r, 16s+4i+j]
    realized via a DRAM bounce: per-round att chunks stream to scratch D
    (cheap 128-partition writes, hidden under compute), then two strided
    D->G reads (SP+Act queues) land the phi1 layout in one shot
  - conv3 reads G with scattered rhs APs; W3a/W3bb are block-diagonal so
    one matmul covers both samples
  - x2 path folded host-side: W3b@(W1b@x+b) = (W3b@W1b)@x + W3b@b
"""
import sys

sys.path.insert(0, "/opt/trn_rl_repo")

import numpy as np

N_CORES = 8
C = 64
H = W = 128
USE_F32R = True

# pooling segments (out_start, count, src_start); same table for H and W
PAIR_SEGS = [(4, 12, 0), (16, 16, 13), (32, 16, 30), (48, 16, 47),
             (64, 16, 64), (80, 16, 81), (96, 16, 98), (112, 12, 115)]
COPY_SEGS = [(0, 4, 0), (124, 4, 127)]  # emitted as self-adds

BLOCK_ROWS = [(0, 30), (30, 64), (64, 98), (98, 128)]

# f32r weight pack columns (matmul weights)
W3BB_O = 0           # block-diag [128,128]
NWR = W3BB_O + 128
# bf16 weight pack columns (block-diag taps + conv3 W3a + conv1a)
WTAP_O = 0           # 9 taps x [128,128]
W3A_O = 9 * 128
W1A_O = W3A_O + 128  # [64,64] dup'd on both partition halves
NWB = W1A_O + 64
# f32 weight pack columns (SE weights + biases)
SE1_O = 0
SE2_O = 8
B1A_O = SE2_O + 64
BPR_O = B1A_O + 1
B3_O = BPR_O + 1
NWF = B3_O + 1

_cache = {}


def _segs(lo, hi):
    out = []
    for (o, c, s) in PAIR_SEGS:
        if lo <= o and o + c <= hi:
            out.append((o, c, s, False))
    for (o, c, s) in COPY_SEGS:
        if lo <= o and o + c <= hi:
            out.append((o, c, s, True))
    return out


def build_program():
    import concourse.bass as bass
    import concourse.tile as tile
    import concourse.mybir as mybir
    from concourse import bacc
    from contextlib import ExitStack

    f32 = mybir.dt.float32
    f32r = mybir.dt.float32r
    Alu = mybir.AluOpType
    Act = mybir.ActivationFunctionType

    bf16 = mybir.dt.bfloat16
    fmm = f32r if USE_F32R else f32

    nc = bacc.Bacc("TRN2", target_bir_lowering=False, debug=False)
    xs_d = nc.dram_tensor("xs", [2, C, H, W], fmm, kind="ExternalInput").ap()
    wr_d = nc.dram_tensor("wpackr", [128, NWR], fmm, kind="ExternalInput").ap()
    wf_d = nc.dram_tensor("wpackf", [128, NWF], f32, kind="ExternalInput").ap()
    wb_d = nc.dram_tensor("wpackb", [128, NWB], mybir.dt.bfloat16,
                          kind="ExternalInput").ap()
    d_d = nc.dram_tensor("foldbuf", [128, 16 * 1024], bf16, kind="Internal").ap()
    out_d = nc.dram_tensor("out", [2, C, H, W], f32, kind="ExternalOutput").ap()

    with tile.TileContext(nc) as tc, ExitStack() as ctx:
        persist = ctx.enter_context(tc.tile_pool(name="persist", bufs=1))
        xh_p = ctx.enter_context(tc.tile_pool(name="xh", bufs=2))
        px_p = ctx.enter_context(tc.tile_pool(name="px", bufs=2))
        prs_p = ctx.enter_context(tc.tile_pool(name="prs", bufs=4))
        att_p = ctx.enter_context(tc.tile_pool(name="att", bufs=2))
        sml_p = ctx.enter_context(tc.tile_pool(name="sml", bufs=4))

        wsr = persist.tile([128, NWR], fmm)
        wsf = persist.tile([128, NWF], f32)
        wsb16 = persist.tile([128, NWB], bf16)
        X = persist.tile([128, H * W], fmm)
        G = persist.tile([128, H * W], bf16)
        pt_ab = [persist.tile([128, 34 * 34], bf16, tag=f"pt{k}",
                              name=f"pt{k}") for k in range(2)]
        X3 = X.rearrange("z (h w) -> z h w", h=H)

        xs_f = xs_d.rearrange("s c h w -> (s c) (h w)")
        # pt zero rings cleared before any input lands (Pool is idle)
        nc.gpsimd.memset(pt_ab[0].bitcast(mybir.dt.float32), 0.0)
        nc.gpsimd.memset(pt_ab[1].bitcast(mybir.dt.float32), 0.0)
        # input blocks + weights spread over the DMA queues so the first
        # pool block and its weights land as early as possible
        r0, r1 = BLOCK_ROWS[0]
        nc.sync.dma_start(out=X[:, r0 * W:r1 * W], in_=xs_f[:, r0 * W:r1 * W])
        nc.sync.dma_start(out=wsb16, in_=wb_d)
        nc.scalar.dma_start(out=wsf, in_=wf_d)
        r0, r1 = BLOCK_ROWS[1]
        nc.scalar.dma_start(out=X[:, r0 * W:r1 * W], in_=xs_f[:, r0 * W:r1 * W])
        r0, r1 = BLOCK_ROWS[2]
        nc.sync.dma_start(out=X[:, r0 * W:r1 * W], in_=xs_f[:, r0 * W:r1 * W])
        r0, r1 = BLOCK_ROWS[3]
        nc.sync.dma_start(out=X[:, r0 * W:r1 * W], in_=xs_f[:, r0 * W:r1 * W])
        nc.sync.dma_start(out=wsr, in_=wr_d)

        with tc.tile_pool(name="psA", bufs=1, space="PSUM") as psA, \
             tc.tile_pool(name="psB", bufs=2, space="PSUM") as psB, \
             tc.tile_pool(name="psSE", bufs=1, space="PSUM") as psSE:

            def tiny_mm(one, dep_ap):
                """single-wait absorber: PE observes dep_ap's producer(s).
                one: (1,1) AP on same partition; dep_ap: (1, ...) region AP."""
                scr = psB.tile([128, 1024], f32, tag="b")
                if one.dtype != f32:
                    one = one.bitcast(f32)
                if dep_ap.dtype != f32:
                    dep_ap = dep_ap.bitcast(f32)
                n = dep_ap.free_size()
                nc.tensor.matmul(scr[0:1, 0:n], one, dep_ap,
                                 start=True, stop=True)

            tiny_mm(wsf[0:1, 0:1], wsb16[0:1, 0:2].bitcast(f32))

            px_tiles = {}

            def do_pool(i, hb):
                xh = xh_p.tile([128, 32 * 64], f32, tag="xh")
                xh3 = xh.rearrange("z (h w) -> z h w", h=32)
                for (o, cnt, s, cp) in _segs(32 * i, 32 * i + 32):
                    ol = o - 32 * i
                    if cp:
                        src = X3[:, s:s + 1, hb * 64:hb * 64 + 64]
                        src = src.broadcast_to((128, cnt, 64))
                        in0 = in1 = src
                    else:
                        in0 = X3[:, s:s + cnt, hb * 64:hb * 64 + 64]
                        in1 = X3[:, s + 1:s + 1 + cnt, hb * 64:hb * 64 + 64]
                    nc.gpsimd.tensor_tensor(
                        out=xh3[:, ol:ol + cnt, :], in0=in0, in1=in1,
                        op=Alu.add)
                px = px_p.tile([128, 2 * 1024], bf16, tag="px")
                px4 = px.rearrange("z (l h w) -> z l h w", l=2, h=32)
                for (o, cnt, s, cp) in _segs(64 * hb, 64 * hb + 64):
                    jloc = (o - 64 * hb) // 32
                    w0 = (o - 64 * hb) % 32
                    sl = s - 64 * hb
                    if cp:
                        src = xh3[:, :, sl:sl + 1].broadcast_to((128, 32, cnt))
                        in0 = in1 = src
                    else:
                        in0 = xh3[:, :, sl:sl + cnt]
                        in1 = xh3[:, :, sl + 1:sl + 1 + cnt]
                    nc.gpsimd.tensor_tensor(
                        out=px4[:, jloc, :, w0:w0 + cnt], in0=in0, in1=in1,
                        op=Alu.add)
                return px

            def stage1(s, t):
                """conv1a + pt evict + taps + prs evict (PE/Act/Pool)."""
                px = px_tiles[(t // 2, t % 2)]
                sb = s * 64
                px4 = px[sb:sb + 64, :].rearrange("z (l h w) -> z l h w",
                                                  l=2, h=32)
                # conv1a -> psA; windows packed on partition halves
                c1 = psA.tile([128, 1024], f32, tag="a")
                for par in range(2):
                    for ch in range(2):
                        nc.tensor.matmul(
                            c1[par * 64:par * 64 + 64,
                               ch * 512:ch * 512 + 512],
                            wsb16[sb:sb + 64, W1A_O:W1A_O + 64],
                            px4[:, par, ch * 16:ch * 16 + 16, :],
                            start=True, stop=True)
                # evict into padded window tile (zero ring, memset once)
                pt = pt_ab[(2 * t + s) % 2]
                pt3 = pt.rearrange("z (a b) -> z a b", a=34)
                c1r = c1.rearrange("z (h w) -> z h w", h=32)
                nc.scalar.activation(
                    out=pt3[0:64, 1:33, 1:33], in_=c1r[0:64],
                    func=Act.Identity, bias=wsf[0:64, B1A_O:B1A_O + 1],
                    scale=1.0)
                nc.vector.tensor_scalar(
                    out=pt3[64:128, 1:33, 1:33], in0=c1r[64:128],
                    scalar1=wsf[64:128, B1A_O:B1A_O + 1], scalar2=None,
                    op0=Alu.add)
                # conv2 + pos taps accumulate; block-diag weights cover both
                # windows (pt partitions 0-63 = win A, 64-127 = win B)
                prp = psB.tile([128, 1024], f32, tag="b")
                for ti in range(9):
                    kh, kw = ti // 3, ti % 3
                    for ch in range(2):
                        nc.tensor.matmul(
                            prp[0:128, ch * 512:ch * 512 + 512],
                            wsb16[0:128, WTAP_O + ti * 128:
                                  WTAP_O + ti * 128 + 128],
                            pt3[0:128,
                                kh + ch * 16:kh + ch * 16 + 16,
                                kw:kw + 32],
                            start=(ti == 0), stop=(ti == 8),
                            skip_group_check=True)
                # evict pr + bias, fused per-window sum (re-stack partitions)
                prs = prs_p.tile([128, 1024], bf16, tag="prs")
                svec = sml_p.tile([128, 1], f32, tag="sv")
                nc.scalar.activation(out=prs, in_=prp[:, 0:1024],
                                     func=Act.Identity,
                                     bias=wsf[:, BPR_O:BPR_O + 1], scale=1.0,
                                     accum_out=svec[:, 0:1])
                return prs, svec

            def stage2(s, t, prs, svec):
                """SE mlp + att + fold-buffer write (PE/Act/DVE/SP)."""
                se1 = psSE.tile([128, 1024], f32, tag="se")
                for par in range(2):
                    pb = par * 64
                    nc.tensor.matmul(se1[0:8, par * 512:par * 512 + 1],
                                     wsf[pb:pb + 64, SE1_O:SE1_O + 8],
                                     svec[pb:pb + 64, 0:1],
                                     start=True, stop=True)
                s1sb = sml_p.tile([128, 1], f32, tag="s1")
                for par in range(2):
                    pb = par * 64
                    nc.vector.tensor_scalar(
                        out=s1sb[pb:pb + 8, 0:1],
                        in0=se1[0:8, par * 512:par * 512 + 1],
                        scalar1=0.0, scalar2=None, op0=Alu.max)
                se2 = psSE.tile([128, 1024], f32, tag="se")
                for par in range(2):
                    pb = par * 64
                    nc.tensor.matmul(se2[0:64, par * 512:par * 512 + 1],
                                     wsf[pb:pb + 8, SE2_O:SE2_O + 64],
                                     s1sb[pb:pb + 8, 0:1],
                                     start=True, stop=True)
                s2sb = sml_p.tile([128, 1], f32, tag="s2")
                for par in range(2):
                    pb = par * 64
                    nc.scalar.activation(out=s2sb[pb:pb + 64, 0:1],
                                         in_=se2[0:64, par * 512:par * 512 + 1],
                                         func=Act.Sigmoid)
                sp = sml_p.tile([128, 1], f32, tag="sp")
                nc.vector.tensor_scalar_add(sp[:, 0:1], s2sb[:, 0:1], 1.0)
                # att = pr * (1 + s); bf16 chunk streamed to the fold buffer
                att = att_p.tile([128, 1024], bf16, tag="att")
                nc.vector.tensor_scalar(out=att, in0=prs, scalar1=sp[:, 0:1],
                                        scalar2=None, op0=Alu.mult)
                deng = nc.scalar if (s == 0 and t == 7) else nc.sync
                deng.dma_start(
                    out=d_d[:, (s * 8 + t) * 1024:(s * 8 + t + 1) * 1024],
                    in_=att)

            # depth-2 software pipeline: stage2(r) is emitted after
            # stage1(r+2) so the SE chain never blocks the in-order queues.
            # pooling runs one block ahead; its PE absorber is emitted at
            # round start so PE never stalls on a future block's pooling.
            from collections import deque
            blocks = [(i, hb) for i in range(4) for hb in range(2)]
            px_tiles[blocks[0]] = do_pool(*blocks[0])
            pending = deque()
            for k, (i, hb) in enumerate(blocks):
                t = i * 2 + hb
                if k + 1 < len(blocks):
                    nb = blocks[k + 1]
                    px_tiles[nb] = do_pool(*nb)
                for s in range(2):
                    r1 = stage1(s, t)
                    pending.append((s, t) + r1)
                    if len(pending) > 2:
                        stage2(*pending.popleft())
            while pending:
                stage2(*pending.popleft())

        # ---- fold: D -> G (phi1 layout), split by q over two queues ----
        # G[s*64+t*8+par*4+b, p*1024+q*512+m] =
        #     D[par*64+b*16+p, (s*8+t)*1024+q*512+m]
        g_q = G.rearrange("z (p q m) -> q z p m", p=16, q=2, m=512)
        d_q = d_d.rearrange("p (st q m) -> q st p m", st=16, q=2, m=512)
        nc.gpsimd.dma_start(out=g_q[0][:, :, 0:256], in_=d_q[0][:, :, 0:256])
        nc.scalar.dma_start(out=g_q[0][:, :, 256:512], in_=d_q[0][:, :, 256:512])
        nc.sync.dma_start(out=g_q[1][:, :, 0:256], in_=d_q[1][:, :, 0:256])
        nc.sync.dma_start(out=g_q[1][:, :, 256:512], in_=d_q[1][:, :, 256:512])

        # ---- conv3 ----
        Xr = X.rearrange("z (i p q w) -> z i p q w", i=4, p=16, q=2)
        Gr = G.rearrange("z (p q r sl ij) -> z p q r sl ij",
                         p=16, q=2, r=16, sl=2)
        od = out_d.rearrange("s c (i p q) w -> (s c) i p q w", i=4, p=16)
        with tc.tile_pool(name="psC", bufs=2, space="PSUM") as psC, \
             tc.tile_pool(name="outp", bufs=4) as out_p:
            for q in range(2):
                for i in range(4):
                    pc = psC.tile([128, 2048], f32, tag="c",
                                  name=f"pc{i}{q}")
                    for j in range(4):
                        nc.tensor.matmul(
                            pc[0:128, j * 512:j * 512 + 512],
                            wsr[0:128, W3BB_O:W3BB_O + 128],
                            Xr[0:128, i, :, q, 32 * j:32 * j + 32],
                            start=True, stop=False)
                    for j in range(4):
                        nc.tensor.matmul(
                            pc[0:128, j * 512:j * 512 + 512],
                            wsb16[0:128, W3A_O:W3A_O + 128],
                            Gr[0:128, :, q, :, :, 4 * i + j],
                            start=False, stop=True)
                    ot = out_p.tile([128, 2048], f32, tag="o")
                    dst = ot.rearrange("z (p j r sl) -> z j p r sl",
                                       p=16, j=4, r=16)
                    src_t = pc[0:64, :].rearrange(
                        "z (j p r sl) -> z j p r sl", j=4, p=16, r=16)
                    src_b = pc[64:128, :].rearrange(
                        "z (j p r sl) -> z j p r sl", j=4, p=16, r=16)
                    # evicts: Act top half, DVE bottom half (Pool can't PSUM)
                    nc.scalar.activation(
                        out=dst[0:64], in_=src_t, func=Act.Identity,
                        bias=wsf[0:64, B3_O:B3_O + 1], scale=1.0)
                    eng_b = nc.scalar if (4 * q + i) == 7 else nc.vector
                    if eng_b is nc.scalar:
                        nc.scalar.activation(
                            out=dst[64:128], in_=src_b, func=Act.Identity,
                            bias=wsf[64:128, B3_O:B3_O + 1], scale=1.0)
                    else:
                        nc.vector.tensor_scalar(
                            out=dst[64:128], in0=src_b,
                            scalar1=wsf[64:128, B3_O:B3_O + 1],
                            scalar2=None, op0=Alu.add)
                    eng = nc.sync if (4 * q + i) % 2 == 0 else nc.gpsimd
                    eng.dma_start(
                        out=od[:, i, :, q, :],
                        in_=ot.rearrange("z (p w) -> z p w", p=16))

    nc.compile()
    return nc


def _prep_inputs(inputs):
    x = np.ascontiguousarray(np.asarray(inputs["x"], dtype=np.float32))
    w1 = np.asarray(inputs["conv1_w"], np.float32)[:, :, 0, 0]
    b1 = np.asarray(inputs["conv1_b"], np.float32)
    w2 = np.asarray(inputs["conv2_w"], np.float32)[:, :, 0, 0]
    b2 = np.asarray(inputs["conv2_b"], np.float32)
    w3 = np.asarray(inputs["conv3_w"], np.float32)[:, :, 0, 0]
    b3 = np.asarray(inputs["conv3_b"], np.float32)
    pw = np.asarray(inputs["pos_w"], np.float32)
    pb = np.asarray(inputs["pos_b"], np.float32)
    s1w = np.asarray(inputs["se_w1"], np.float32)[:, :, 0, 0]
    s2w = np.asarray(inputs["se_w2"], np.float32)[:, :, 0, 0]

    wpr = np.zeros((128, NWR), np.float32)
    wpf = np.zeros((128, NWF), np.float32)

    def dup(dst, col, mat):
        dst[0:mat.shape[0], col:col + mat.shape[1]] = mat
        dst[64:64 + mat.shape[0], col:col + mat.shape[1]] = mat

    def bdiag(dst, col, mat):
        dst[0:64, col:col + 64] = mat
        dst[64:128, col + 64:col + 128] = mat

    W3a, W3b = w3[:, :64], w3[:, 64:]
    bdiag(wpr, W3BB_O, (W3b @ w1[64:]).T)
    dup(wpf, SE1_O, (s1w / 1024.0).T)
    dup(wpf, SE2_O, s2w.T)
    dup(wpf, B1A_O, b1[:64][:, None])
    dup(wpf, BPR_O, (b2 + pb)[:, None])
    dup(wpf, B3_O, (b3 + W3b @ b1[64:])[:, None])
    import ml_dtypes
    wpb = np.zeros((128, NWB), ml_dtypes.bfloat16)
    for kh in range(3):
        for kw in range(3):
            tap = pw[:, :, kh, kw]
            if kh == 1 and kw == 1:
                tap = tap + w2
            t16 = tap.T.astype(ml_dtypes.bfloat16)
            k = WTAP_O + (kh * 3 + kw) * 128
            wpb[0:64, k:k + 64] = t16
            wpb[64:128, k + 64:k + 128] = t16
    w3a16 = W3a.T.astype(ml_dtypes.bfloat16)
    wpb[0:64, W3A_O:W3A_O + 64] = w3a16
    wpb[64:128, W3A_O + 64:W3A_O + 128] = w3a16
    w1a16 = (0.25 * w1[:64]).T.astype(ml_dtypes.bfloat16)
    wpb[0:64, W1A_O:W1A_O + 64] = w1a16
    wpb[64:128, W1A_O:W1A_O + 64] = w1a16
    return x, wpr, wpf, wpb


def kernel(**inputs):
    from concourse.bass_utils import run_bass_kernel_spmd

    if "nc" not in _cache:
        _cache["nc"] = build_program()
    nc = _cache["nc"]
    x, wpr, wpf, wpb = _prep_inputs(inputs)
    n = x.shape[0]
    per = n // N_CORES
    in_maps = [{"xs": x[c * per:(c + 1) * per], "wpackr": wpr, "wpackf": wpf,
                "wpackb": wpb} for c in range(N_CORES)]
    res = run_bass_kernel_spmd(nc, in_maps, list(range(N_CORES)))
    _cache["last_res"] = res
    out = np.concatenate([res.results[c]["out"] for c in range(N_CORES)], axis=0)
    return out.astype(np.float32)


# revision 24
# speedup vs baseline: 1.2405x; 1.2405x over previous
"""Trainium2 Bass kernel for nn_DPConv_16011638080100.

8 NeuronCores, data-parallel over batch N=16 -> 2 samples/core.

Per core:
  - pooling commutes with 1x1 conv1: pool x first (separable 2-tap averages;
    edge-copy rows done as self-adds so a uniform 0.25 folds into W1a)
  - conv1a on pooled windows -> zero-ring-padded window tiles
  - conv2 + pos 3x3 as 9 PSUM-accumulated matmul taps (center fused w/ conv2);
    tap weights are block-diagonal [128,128] so one matmul covers both
    windows of the round (pt holds window A on partitions 0-63, B on 64-127)
  - SE reduce fused into the PSUM eviction via activation(accum_out=...)
  - torch-.view fold == index permutation
      out[4a+b, 32i+2p+q, 32j+2r+s] = att_a[16b+p, 16q+r, 16s+4i+j]
    realized via a DRAM bounce: per-round att chunks stream to scratch D
    (cheap 128-partition writes, hidden under compute), then two strided
    D->G reads (SP+Act queues) land the phi1 layout in one shot
  - conv3 reads G with scattered rhs APs; W3a/W3bb are block-diagonal so
    one matmul covers both samples
  - x2 path folded host-side: W3b@(W1b@x+b) = (W3b@W1b)@x + W3b@b
"""
import sys

sys.path.insert(0, "/opt/trn_rl_repo")

import numpy as np

N_CORES = 8
C = 64
H = W = 128
USE_F32R = True

# pooling segments (out_start, count, src_start); same table for H and W
PAIR_SEGS = [(4, 12, 0), (16, 16, 13), (32, 16, 30), (48, 16, 47),
             (64, 16, 64), (80, 16, 81), (96, 16, 98), (112, 12, 115)]
COPY_SEGS = [(0, 4, 0), (124, 4, 127)]  # emitted as self-adds

BLOCK_ROWS = [(0, 30), (30, 64), (64, 98), (98, 128)]

# f32r weight pack columns (matmul weights)
W3BB_O = 0           # block-diag [128,128]
NWR = W3BB_O + 128
# bf16 weight pack columns (block-diag taps + conv3 W3a + conv1a)
WTAP_O = 0           # 9 taps x [128,128]
W3A_O = 9 * 128
W1A_O = W3A_O + 128  # [64,64] dup'd on both partition halves
NWB = W1A_O + 64
# f32 weight pack columns (SE weights + biases)
SE1_O = 0
SE2_O = 8
B1A_O = SE2_O + 64
BPR_O = B1A_O + 1
B3_O = BPR_O + 1
NWF = B3_O + 1

_cache = {}


def _segs(lo, hi):
    out = []
    for (o, c, s) in PAIR_SEGS:
        if lo <= o and o + c <= hi:
            out.append((o, c, s, False))
    for (o, c, s) in COPY_SEGS:
        if lo <= o and o + c <= hi:
            out.append((o, c, s, True))
    return out


def build_program():
    import concourse.bass as bass
    import concourse.tile as tile
    import concourse.mybir as mybir
    from concourse import bacc
    from contextlib import ExitStack

    f32 = mybir.dt.float32
    f32r = mybir.dt.float32r
    Alu = mybir.AluOpType
    Act = mybir.ActivationFunctionType

    bf16 = mybir.dt.bfloat16
    fmm = f32r if USE_F32R else f32

    nc = bacc.Bacc("TRN2", target_bir_lowering=False, debug=False)
    xs_d = nc.dram_tensor("xs", [2, C, H, W], fmm, kind="ExternalInput").ap()
    wr_d = nc.dram_tensor("wpackr", [128, NWR], fmm, kind="ExternalInput").ap()
    wf_d = nc.dram_tensor("wpackf", [128, NWF], f32, kind="ExternalInput").ap()
    wb_d = nc.dram_tensor("wpackb", [128, NWB], mybir.dt.bfloat16,
                          kind="ExternalInput").ap()
    d_d = nc.dram_tensor("foldbuf", [128, 16 * 1024], bf16, kind="Internal").ap()
    out_d = nc.dram_tensor("out", [2, C, H, W], f32, kind="ExternalOutput").ap()

    with tile.TileContext(nc) as tc, ExitStack() as ctx:
        persist = ctx.enter_context(tc.tile_pool(name="persist", bufs=1))
        xh_p = ctx.enter_context(tc.tile_pool(name="xh", bufs=2))
        px_p = ctx.enter_context(tc.tile_pool(name="px", bufs=2))
        prs_p = ctx.enter_context(tc.tile_pool(name="prs", bufs=4))
        att_p = ctx.enter_context(tc.tile_pool(name="att", bufs=2))
        sml_p = ctx.enter_context(tc.tile_pool(name="sml", bufs=4))

        wsr = persist.tile([128, NWR], fmm)
        wsf = persist.tile([128, NWF], f32)
        wsb16 = persist.tile([128, NWB], bf16)
        X = persist.tile([128, H * W], fmm)
        G = persist.tile([128, H * W], bf16)
        pt_ab = [persist.tile([128, 34 * 34], bf16, tag=f"pt{k}",
                              name=f"pt{k}") for k in range(2)]
        X3 = X.rearrange("z (h w) -> z h w", h=H)

        xs_f = xs_d.rearrange("s c h w -> (s c) (h w)")
        # pt zero rings cleared before any input lands (Pool is idle)
        nc.gpsimd.memset(pt_ab[0].bitcast(mybir.dt.float32), 0.0)
        nc.gpsimd.memset(pt_ab[1].bitcast(mybir.dt.float32), 0.0)
        # input blocks + weights spread over the DMA queues so the first
        # pool block and its weights land as early as possible
        r0, r1 = BLOCK_ROWS[0]
        nc.sync.dma_start(out=X[:, r0 * W:r1 * W], in_=xs_f[:, r0 * W:r1 * W])
        nc.sync.dma_start(out=wsb16, in_=wb_d)
        nc.scalar.dma_start(out=wsf, in_=wf_d)
        r0, r1 = BLOCK_ROWS[1]
        nc.scalar.dma_start(out=X[:, r0 * W:r1 * W], in_=xs_f[:, r0 * W:r1 * W])
        r0, r1 = BLOCK_ROWS[2]
        nc.sync.dma_start(out=X[:, r0 * W:r1 * W], in_=xs_f[:, r0 * W:r1 * W])
        r0, r1 = BLOCK_ROWS[3]
        nc.sync.dma_start(out=X[:, r0 * W:r1 * W], in_=xs_f[:, r0 * W:r1 * W])
        nc.sync.dma_start(out=wsr, in_=wr_d)

        with tc.tile_pool(name="psA", bufs=2, space="PSUM") as psA, \
             tc.tile_pool(name="psB", bufs=1, space="PSUM") as psB, \
             tc.tile_pool(name="psSE", bufs=1, space="PSUM") as psSE:

            def tiny_mm(one, dep_ap):
                """single-wait absorber: PE observes dep_ap's producer(s).
                one: (1,1) AP on same partition; dep_ap: (1, ...) region AP."""
                scr = psB.tile([128, 1024], f32, tag="b")
                if one.dtype != f32:
                    one = one.bitcast(f32)
                if dep_ap.dtype != f32:
                    dep_ap = dep_ap.bitcast(f32)
                n = dep_ap.free_size()
                nc.tensor.matmul(scr[0:1, 0:n], one, dep_ap,
                                 start=True, stop=True)

            tiny_mm(wsf[0:1, 0:1], wsb16[0:1, 0:2].bitcast(f32))

            px_tiles = {}

            def do_pool(i, hb):
                xh = xh_p.tile([128, 32 * 64], f32, tag="xh")
                xh3 = xh.rearrange("z (h w) -> z h w", h=32)
                for (o, cnt, s, cp) in _segs(32 * i, 32 * i + 32):
                    ol = o - 32 * i
                    if cp:
                        src = X3[:, s:s + 1, hb * 64:hb * 64 + 64]
                        src = src.broadcast_to((128, cnt, 64))
                        in0 = in1 = src
                    else:
                        in0 = X3[:, s:s + cnt, hb * 64:hb * 64 + 64]
                        in1 = X3[:, s + 1:s + 1 + cnt, hb * 64:hb * 64 + 64]
                    nc.gpsimd.tensor_tensor(
                        out=xh3[:, ol:ol + cnt, :], in0=in0, in1=in1,
                        op=Alu.add)
                px = px_p.tile([128, 2 * 1024], bf16, tag="px")
                px4 = px.rearrange("z (l h w) -> z l h w", l=2, h=32)
                for (o, cnt, s, cp) in _segs(64 * hb, 64 * hb + 64):
                    jloc = (o - 64 * hb) // 32
                    w0 = (o - 64 * hb) % 32
                    sl = s - 64 * hb
                    if cp:
                        src = xh3[:, :, sl:sl + 1].broadcast_to((128, 32, cnt))
                        in0 = in1 = src
                    else:
                        in0 = xh3[:, :, sl:sl + cnt]
                        in1 = xh3[:, :, sl + 1:sl + 1 + cnt]
                    nc.gpsimd.tensor_tensor(
                        out=px4[:, jloc, :, w0:w0 + cnt], in0=in0, in1=in1,
                        op=Alu.add)
                return px

            def stage1(s, t):
                """conv1a + pt evict + taps + prs evict (PE/Act/Pool)."""
                px = px_tiles[(t // 2, t % 2)]
                sb = s * 64
                px4 = px[sb:sb + 64, :].rearrange("z (l h w) -> z l h w",
                                                  l=2, h=32)
                # conv1a -> psA; windows packed on partition halves
                c1 = psA.tile([128, 1024], f32, tag="a")
                for par in range(2):
                    for ch in range(2):
                        nc.tensor.matmul(
                            c1[par * 64:par * 64 + 64,
                               ch * 512:ch * 512 + 512],
                            wsb16[sb:sb + 64, W1A_O:W1A_O + 64],
                            px4[:, par, ch * 16:ch * 16 + 16, :],
                            start=True, stop=True)
                # evict into padded window tile (zero ring, memset once)
                pt = pt_ab[(2 * t + s) % 2]
                pt3 = pt.rearrange("z (a b) -> z a b", a=34)
                c1r = c1.rearrange("z (h w) -> z h w", h=32)
                nc.scalar.activation(
                    out=pt3[0:64, 1:33, 1:33], in_=c1r[0:64],
                    func=Act.Identity, bias=wsf[0:64, B1A_O:B1A_O + 1],
                    scale=1.0)
                nc.vector.tensor_scalar(
                    out=pt3[64:128, 1:33, 1:33], in0=c1r[64:128],
                    scalar1=wsf[64:128, B1A_O:B1A_O + 1], scalar2=None,
                    op0=Alu.add)
                # conv2 + pos taps accumulate; block-diag weights cover both
                # windows (pt partitions 0-63 = win A, 64-127 = win B)
                prp = psB.tile([128, 1024], f32, tag="b")
                for ti in range(9):
                    kh, kw = ti // 3, ti % 3
                    for ch in range(2):
                        nc.tensor.matmul(
                            prp[0:128, ch * 512:ch * 512 + 512],
                            wsb16[0:128, WTAP_O + ti * 128:
                                  WTAP_O + ti * 128 + 128],
                            pt3[0:128,
                                kh + ch * 16:kh + ch * 16 + 16,
                                kw:kw + 32],
                            start=(ti == 0), stop=(ti == 8),
                            skip_group_check=True)
                # evict pr + bias, fused per-window sum (re-stack partitions)
                prs = prs_p.tile([128, 1024], bf16, tag="prs")
                svec = sml_p.tile([128, 1], f32, tag="sv")
                nc.scalar.activation(out=prs, in_=prp[:, 0:1024],
                                     func=Act.Identity,
                                     bias=wsf[:, BPR_O:BPR_O + 1], scale=1.0,
                                     accum_out=svec[:, 0:1])
                return prs, svec

            def stage2(s, t, prs, svec):
                """SE mlp + att + fold-buffer write (PE/Act/DVE/SP)."""
                se1 = psSE.tile([128, 1024], f32, tag="se")
                for par in range(2):
                    pb = par * 64
                    nc.tensor.matmul(se1[0:8, par * 512:par * 512 + 1],
                                     wsf[pb:pb + 64, SE1_O:SE1_O + 8],
                                     svec[pb:pb + 64, 0:1],
                                     start=True, stop=True)
                s1sb = sml_p.tile([128, 1], f32, tag="s1")
                for par in range(2):
                    pb = par * 64
                    nc.vector.tensor_scalar(
                        out=s1sb[pb:pb + 8, 0:1],
                        in0=se1[0:8, par * 512:par * 512 + 1],
                        scalar1=0.0, scalar2=None, op0=Alu.max)
                se2 = psSE.tile([128, 1024], f32, tag="se")
                for par in range(2):
                    pb = par * 64
                    nc.tensor.matmul(se2[0:64, par * 512:par * 512 + 1],
                                     wsf[pb:pb + 8, SE2_O:SE2_O + 64],
                                     s1sb[pb:pb + 8, 0:1],
                                     start=True, stop=True)
                s2sb = sml_p.tile([128, 1], f32, tag="s2")
                for par in range(2):
                    pb = par * 64
                    nc.scalar.activation(out=s2sb[pb:pb + 64, 0:1],
                                         in_=se2[0:64, par * 512:par * 512 + 1],
                                         func=Act.Sigmoid)
                sp = sml_p.tile([128, 1], f32, tag="sp")
                nc.vector.tensor_scalar_add(sp[:, 0:1], s2sb[:, 0:1], 1.0)
                # att = pr * (1 + s); bf16 chunk streamed to the fold buffer
                att = att_p.tile([128, 1024], bf16, tag="att")
                nc.vector.tensor_scalar(out=att, in0=prs, scalar1=sp[:, 0:1],
                                        scalar2=None, op0=Alu.mult)
                deng = nc.scalar if (s == 0 and t == 7) else nc.sync
                deng.dma_start(
                    out=d_d[:, (s * 8 + t) * 1024:(s * 8 + t + 1) * 1024],
                    in_=att)

            # depth-2 software pipeline: stage2(r) is emitted after
            # stage1(r+2) so the SE chain never blocks the in-order queues.
            # pooling runs one block ahead; its PE absorber is emitted at
            # round start so PE never stalls on a future block's pooling.
            from collections import deque
            blocks = [(i, hb) for i in range(4) for hb in range(2)]
            px_tiles[blocks[0]] = do_pool(*blocks[0])
            pending = deque()
            for k, (i, hb) in enumerate(blocks):
                t = i * 2 + hb
                if k + 1 < len(blocks):
                    nb = blocks[k + 1]
                    px_tiles[nb] = do_pool(*nb)
                for s in range(2):
                    r1 = stage1(s, t)
                    pending.append((s, t) + r1)
                    if len(pending) > 2:
                        stage2(*pending.popleft())
            while pending:
                stage2(*pending.popleft())

        # ---- fold: D -> G (phi1 layout), split by q over two queues ----
        # G[s*64+t*8+par*4+b, p*1024+q*512+m] =
        #     D[par*64+b*16+p, (s*8+t)*1024+q*512+m]
        g_q = G.rearrange("z (p q m) -> q z p m", p=16, q=2, m=512)
        d_q = d_d.rearrange("p (st q m) -> q st p m", st=16, q=2, m=512)
        nc.gpsimd.dma_start(out=g_q[0][:, :, 0:256], in_=d_q[0][:, :, 0:256])
        nc.scalar.dma_start(out=g_q[0][:, :, 256:512], in_=d_q[0][:, :, 256:512])
        nc.sync.dma_start(out=g_q[1][:, :, 0:256], in_=d_q[1][:, :, 0:256])
        nc.sync.dma_start(out=g_q[1][:, :, 256:512], in_=d_q[1][:, :, 256:512])

        # ---- conv3 ----
        Xr = X.rearrange("z (i p q w) -> z i p q w", i=4, p=16, q=2)
        Gr = G.rearrange("z (p q r sl ij) -> z p q r sl ij",
                         p=16, q=2, r=16, sl=2)
        od = out_d.rearrange("s c (i p q) w -> (s c) i p q w", i=4, p=16)
        with tc.tile_pool(name="psC", bufs=2, space="PSUM") as psC, \
             tc.tile_pool(name="outp", bufs=4) as out_p:
            for q in range(2):
                for i in range(4):
                    pc = psC.tile([128, 2048], f32, tag="c",
                                  name=f"pc{i}{q}")
                    for j in range(4):
                        nc.tensor.matmul(
                            pc[0:128, j * 512:j * 512 + 512],
                            wsr[0:128, W3BB_O:W3BB_O + 128],
                            Xr[0:128, i, :, q, 32 * j:32 * j + 32],
                            start=True, stop=False)
                    for j in range(4):
                        nc.tensor.matmul(
                            pc[0:128, j * 512:j * 512 + 512],
                            wsb16[0:128, W3A_O:W3A_O + 128],
                            Gr[0:128, :, q, :, :, 4 * i + j],
                            start=False, stop=True)
                    ot = out_p.tile([128, 2048], f32, tag="o")
                    dst = ot.rearrange("z (p j r sl) -> z j p r sl",
                                       p=16, j=4, r=16)
                    src_t = pc[0:64, :].rearrange(
                        "z (j p r sl) -> z j p r sl", j=4, p=16, r=16)
                    src_b = pc[64:128, :].rearrange(
                        "z (j p r sl) -> z j p r sl", j=4, p=16, r=16)
                    # evicts: Act top half, DVE bottom half (Pool can't PSUM)
                    nc.scalar.activation(
                        out=dst[0:64], in_=src_t, func=Act.Identity,
                        bias=wsf[0:64, B3_O:B3_O + 1], scale=1.0)
                    eng_b = nc.scalar if (4 * q + i) == 7 else nc.vector
                    if eng_b is nc.scalar:
                        nc.scalar.activation(
                            out=dst[64:128], in_=src_b, func=Act.Identity,
                            bias=wsf[64:128, B3_O:B3_O + 1], scale=1.0)
                    else:
                        nc.vector.tensor_scalar(
                            out=dst[64:128], in0=src_b,
                            scalar1=wsf[64:128, B3_O:B3_O + 1],
                            scalar2=None, op0=Alu.add)
                    eng = nc.sync if (4 * q + i) % 2 == 0 else nc.gpsimd
                    eng.dma_start(
                        out=od[:, i, :, q, :],
                        in_=ot.rearrange("z (p w) -> z p w", p=16))

    nc.compile()
    return nc


def _prep_inputs(inputs):
    x = np.ascontiguousarray(np.asarray(inputs["x"], dtype=np.float32))
    w1 = np.asarray(inputs["conv1_w"], np.float32)[:, :, 0, 0]
    b1 = np.asarray(inputs["conv1_b"], np.float32)
    w2 = np.asarray(inputs["conv2_w"], np.float32)[:, :, 0, 0]
    b2 = np.asarray(inputs["conv2_b"], np.float32)
    w3 = np.asarray(inputs["conv3_w"], np.float32)[:, :, 0, 0]
    b3 = np.asarray(inputs["conv3_b"], np.float32)
    pw = np.asarray(inputs["pos_w"], np.float32)
    pb = np.asarray(inputs["pos_b"], np.float32)
    s1w = np.asarray(inputs["se_w1"], np.float32)[:, :, 0, 0]
    s2w = np.asarray(inputs["se_w2"], np.float32)[:, :, 0, 0]

    wpr = np.zeros((128, NWR), np.float32)
    wpf = np.zeros((128, NWF), np.float32)

    def dup(dst, col, mat):
        dst[0:mat.shape[0], col:col + mat.shape[1]] = mat
        dst[64:64 + mat.shape[0], col:col + mat.shape[1]] = mat

    def bdiag(dst, col, mat):
        dst[0:64, col:col + 64] = mat
        dst[64:128, col + 64:col + 128] = mat

    W3a, W3b = w3[:, :64], w3[:, 64:]
    bdiag(wpr, W3BB_O, (W3b @ w1[64:]).T)
    dup(wpf, SE1_O, (s1w / 1024.0).T)
    dup(wpf, SE2_O, s2w.T)
    dup(wpf, B1A_O, b1[:64][:, None])
    dup(wpf, BPR_O, (b2 + pb)[:, None])
    dup(wpf, B3_O, (b3 + W3b @ b1[64:])[:, None])
    import ml_dtypes
    wpb = np.zeros((128, NWB), ml_dtypes.bfloat16)
    for kh in range(3):
        for kw in range(3):
            tap = pw[:, :, kh, kw]
            if kh == 1 and kw == 1:
                tap = tap + w2
            t16 = tap.T.astype(ml_dtypes.bfloat16)
            k = WTAP_O + (kh * 3 + kw) * 128
            wpb[0:64, k:k + 64] = t16
            wpb[64:128, k + 64:k + 128] = t16
    w3a16 = W3a.T.astype(ml_dtypes.bfloat16)
    wpb[0:64, W3A_O:W3A_O + 64] = w3a16
    wpb[64:128, W3A_O + 64:W3A_O + 128] = w3a16
    w1a16 = (0.25 * w1[:64]).T.astype(ml_dtypes.bfloat16)
    wpb[0:64, W1A_O:W1A_O + 64] = w1a16
    wpb[64:128, W1A_O:W1A_O + 64] = w1a16
    return x, wpr, wpf, wpb


def kernel(**inputs):
    from concourse.bass_utils import run_bass_kernel_spmd

    if "nc" not in _cache:
        _cache["nc"] = build_program()
    nc = _cache["nc"]
    x, wpr, wpf, wpb = _prep_inputs(inputs)
    n = x.shape[0]
    per = n // N_CORES
    in_maps = [{"xs": x[c * per:(c + 1) * per], "wpackr": wpr, "wpackf": wpf,
                "wpackb": wpb} for c in range(N_CORES)]
    res = run_bass_kernel_spmd(nc, in_maps, list(range(N_CORES)))
    _cache["last_res"] = res
    out = np.concatenate([res.results[c]["out"] for c in range(N_CORES)], axis=0)
    return out.astype(np.float32)
